# revision 11
# baseline (speedup 1.0000x reference)
"""Two-layer GAT (GATConv x2, PyG-style with self-loops) on 8 Trainium2 cores.

Strategy (dst-sharded, per the problem's sharding hint):
  - Nodes are sharded across 8 cores (12544 nodes/core, padded from 100000).
  - Every core computes the full first-layer node transform
    hext1 = [x@W1 | a_src | a_dst] for all nodes (duplicated compute is
    cheaper than communicating it), writes it to its own HBM.
  - Edges (with self-loops) are bucketed by destination 128-node block and
    padded to a uniform tiles-per-block so one SPMD program serves all cores.
  - Per edge tile (128 edges): indirect-DMA gather of hext[src] rows, a
    selection matrix S01[e,d] = (dst_e == d) built with one is_equal, then
    PSUM-accumulated matmuls compute both the softmax numerator
    sum_e exp(e_e) * h[src_e] and denominator sum_e exp(e_e) per dst node.
    Softmax max-subtraction is skipped: logits are O(5) so exp() is safe in
    fp32, and softmax is shift-invariant so the result is identical.
  - Layer-1 block outputs are ELU'd, transposed, AllGathered (h1^T shards),
    then layer 2 repeats the same pipeline with W2.

kernel() takes full inputs, returns the full [100000, 64] output.
"""
import os
import sys
from contextlib import ExitStack

import numpy as np

# ---------------- problem constants (hardcoded per harness contract) -------
N = 100000
NCORES = 8
P = 128
F_IN = 128
H1 = 2
C1 = 64
HC1 = 128          # H1*C1
C2 = 64
NS = 12544         # nodes per core shard = 98 * 128
B = NS // P        # 98 dst blocks per core
NPAD = NS * NCORES # 100352
W1C = HC1 + 2 * H1 # 132 = [h | a_s(2) | a_d(2)]
W2C = C2 + 2       # 66  = [h | a_s(1) | a_d(1)]
NEG_SLOPE = 0.2
DEN_EPS = 1e-30

_SHIM = os.path.join(os.path.dirname(os.path.abspath(__file__)), "shim")


def _ensure_axon_hooks():
    """bass_utils' trace path needs antenv.axon_hooks; provide it if absent."""
    try:
        import antenv.axon_hooks  # noqa: F401
        return
    except ImportError:
        pass
    import types
    import antenv
    mod = types.ModuleType("antenv.axon_hooks")
    mod._hook = None
    def set_axon_ntff_profile_hook(hook):
        mod._hook = hook
    def get_axon_ntff_profile_hook():
        return mod._hook
    mod.set_axon_ntff_profile_hook = set_axon_ntff_profile_hook
    mod.get_axon_ntff_profile_hook = get_axon_ntff_profile_hook
    sys.modules["antenv.axon_hooks"] = mod
    antenv.axon_hooks = mod


# ---------------- host-side preprocessing ----------------------------------
def _att_mat(att_src, att_dst, cin):
    """Block-diagonal [cin, H] matrices so a_s = h @ As, a_d = h @ Ad."""
    h, c = att_src.shape
    As = np.zeros((cin, h), np.float32)
    Ad = np.zeros((cin, h), np.float32)
    for i in range(h):
        As[i * c:(i + 1) * c, i] = att_src[i]
        Ad[i * c:(i + 1) * c, i] = att_dst[i]
    return As, Ad


def _prep_edges(edge_index):
    """Bucket self-loop-augmented edges by destination 128-block; pad each
    block to a uniform T_B tiles of 128 edge slots. Returns per-core SBUF
    layouts: src indices [8,128,NT] int32, dst offsets [8,128,NT] f32, and
    T_B. Padding slots have dst -1 (never matches) and src 0."""
    # self-loops are NOT added here: the per-block a_d gather (adg) already
    # holds each block's own node rows, so the device adds the self-loop
    # contribution with an identity selection matrix at zero gather cost.
    src = np.asarray(edge_index[0], np.int64)
    dst = np.asarray(edge_index[1], np.int64)
    order = np.argsort(dst, kind="stable")
    src, dst = src[order], dst[order]
    nblk = NPAD // P  # 784
    blk = (dst // P).astype(np.int64)
    bc = np.bincount(blk, minlength=nblk)
    t_b = int(-(-bc.max() // P))
    tbe = t_b * P
    src_slot = np.zeros((nblk, tbe), np.int32)
    dst_slot = np.full((nblk, tbe), -1.0, np.float32)
    starts = np.zeros(nblk + 1, np.int64)
    np.cumsum(bc, out=starts[1:])
    pos = np.arange(len(dst)) - starts[blk]
    src_slot[blk, pos] = src
    dst_slot[blk, pos] = (dst % P).astype(np.float32)
    nt = B * t_b
    # [core, block, tile, slot] -> SBUF layout [core, partition=slot, block*T_B+tile]
    src_tiles = src_slot.reshape(NCORES, B, t_b, P).transpose(0, 3, 1, 2).reshape(NCORES, P, nt)
    dst_tiles = dst_slot.reshape(NCORES, B, t_b, P).transpose(0, 3, 1, 2).reshape(NCORES, P, nt)
    return np.ascontiguousarray(src_tiles), np.ascontiguousarray(dst_tiles), t_b


# ---------------- bass program --------------------------------------------
def _build_program(t_b):
    import concourse.bass as bass
    import concourse.tile as tile
    from concourse import mybir
    from concourse.vector_clock import ScopedClock

    f32 = mybir.dt.float32
    i32 = mybir.dt.int32
    Act = mybir.ActivationFunctionType
    Alu = mybir.AluOpType
    nt = B * t_b

    class PatchedTileContext(tile.TileContext):
        """Kernel-tail drain must not carry more waits than the ISA allows;
        split them across chained drains (this walrus allows 1 wait/inst)."""
        def _drain_and_barrier(self, tick_clock, wait_clock):
            drain_inst = self.nc.sync.drain()
            wait_clock.add_sem_waits(
                drain_inst.ins, ScopedClock({None: tick_clock.global_clock})
            )
            si = drain_inst.ins.sync_info
            if si is not None and si.on_wait and len(si.on_wait) > 1:
                waits = list(si.on_wait)
                si.on_wait = waits[:1]
                rest = waits[1:]
                while rest:
                    extra = self.nc.sync.drain()
                    extra.ins.sync_info = mybir.SyncInfo(on_wait=rest[:1], on_update=[])
                    rest = rest[1:]
            self.nc.all_engine_barrier()
            assert self.sems is not None
            popped = self.nc._tile_sem_poison_stack.pop()
            assert popped is self._sem_poison
            self.nc.clear_and_free_semaphores(list(self.sems.allocated().values()))
            self.nc.all_engine_barrier()

    nc = bass.Bass(num_devices=NCORES)

    xT = nc.declare_dram_parameter("xT", [P, NPAD], f32, isOutput=False)
    w1cat = nc.declare_dram_parameter("w1cat", [P, W1C], f32, isOutput=False)
    w2cat = nc.declare_dram_parameter("w2cat", [P, W2C], f32, isOutput=False)
    b1row = nc.declare_dram_parameter("b1row", [1, HC1], f32, isOutput=False)
    b2row = nc.declare_dram_parameter("b2row", [1, C2], f32, isOutput=False)
    iota_in = nc.declare_dram_parameter("iota_rows", [P, P], f32, isOutput=False)
    ident_in = nc.declare_dram_parameter("ident", [P, P], f32, isOutput=False)
    srcidx_in = nc.declare_dram_parameter("srcidx", [P, nt], i32, isOutput=False)
    dstcol_in = nc.declare_dram_parameter("dstcol", [P, nt], f32, isOutput=False)
    adidx_in = nc.declare_dram_parameter("adidx", [P, B], i32, isOutput=False)
    out2 = nc.declare_dram_parameter("out2", [NS, C2], f32, isOutput=True)

    with PatchedTileContext(nc) as tc, ExitStack() as ctx:
        const = ctx.enter_context(tc.tile_pool(name="const", bufs=1))
        dram = ctx.enter_context(tc.tile_pool(name="dram", bufs=1, space="DRAM"))

        hext1 = dram.tile([NPAD, W1C], f32)
        hext2 = dram.tile([NPAD, P], f32)  # W2C cols used; row padded to 512B
        h1t_shard = dram.tile([P, NS], f32)
        h1t_full = dram.tile([NCORES * P, NS], f32, addr_space="Shared")

        # resident constants / index tables
        w1_sb = const.tile([P, W1C], f32)
        nc.sync.dma_start(out=w1_sb[:], in_=w1cat[:])
        w2_sb = const.tile([P, W2C], f32)
        nc.sync.dma_start(out=w2_sb[:], in_=w2cat[:])
        iota_sb = const.tile([P, P], f32)
        nc.sync.dma_start(out=iota_sb[:], in_=iota_in[:])
        ident_sb = const.tile([P, P], f32)
        nc.sync.dma_start(out=ident_sb[:], in_=ident_in[:])
        b1_sb = const.tile([P, HC1], f32)
        nc.sync.dma_start(out=b1_sb[:], in_=b1row[0:1, :].to_broadcast([P, HC1]))
        b2_sb = const.tile([P, C2], f32)
        nc.sync.dma_start(out=b2_sb[:], in_=b2row[0:1, :].to_broadcast([P, C2]))
        srcidx_sb = const.tile([P, nt], i32)
        nc.sync.dma_start(out=srcidx_sb[:], in_=srcidx_in[:])
        dstcol_sb = const.tile([P, nt], f32)
        nc.sync.dma_start(out=dstcol_sb[:], in_=dstcol_in[:])
        adidx_sb = const.tile([P, B], i32)
        nc.sync.dma_start(out=adidx_sb[:], in_=adidx_in[:])

        def phase1(src_view, wcat_sb, wcols, hext, slab_tiles, n_slabs, store_cols):
            """hext[n,:] = xT_tile.T @ wcat for all node tiles. store_cols is
            the hext row width (>= wcols; padded so DMA descriptors are
            >=512B and avoid the SDMA read-modify-write penalty)."""
            with ExitStack() as c2:
                sbp = c2.enter_context(tc.tile_pool(name="p1sb", bufs=3))
                psp = c2.enter_context(tc.tile_pool(name="p1ps", bufs=3, space="PSUM"))
                for s in range(n_slabs):
                    w = slab_tiles * P
                    slab = sbp.tile([P, w], f32, tag="slab")
                    nc.sync.dma_start(out=slab[:], in_=src_view(s))
                    for k in range(slab_tiles):
                        i = s * slab_tiles + k
                        ps = psp.tile([P, wcols], f32, tag="ps")
                        nc.tensor.matmul(
                            out=ps[:], lhsT=slab[:, k * P:(k + 1) * P],
                            rhs=wcat_sb[:], start=True, stop=True,
                        )
                        he = sbp.tile([P, store_cols], f32, tag="he")
                        nc.vector.tensor_copy(out=he[:, 0:wcols], in_=ps[:])
                        nc.sync.dma_start(
                            out=hext[i * P:(i + 1) * P, :], in_=he[:]
                        )

        def edge_phase(hext, gwidth, heads, cdim, bias_sb, layer1):
            """Per dst block: accumulate softmax numerator/denominator over
            edge tiles, normalize, then store (L1: ELU + transpose to h1T
            shard; L2: final output rows)."""
            wcols = gwidth
            scol = heads * cdim           # a_src column offset in hext row
            ncols = scol + heads          # matmul rhs width = msg | ex
            with ExitStack() as c2:
                sbe = c2.enter_context(tc.tile_pool(name="esb", bufs=8))
                sbs = c2.enter_context(tc.tile_pool(name="esmall", bufs=6))
                pso = c2.enter_context(tc.tile_pool(name="epso", bufs=2, space="PSUM"))
                pst = c2.enter_context(tc.tile_pool(name="epst", bufs=2, space="PSUM"))
                pse = c2.enter_context(tc.tile_pool(name="epse", bufs=3, space="PSUM"))
                def issue_adg(bb):
                    t = sbe.tile([P, wcols], f32, tag="adg")
                    nc.gpsimd.indirect_dma_start(
                        out=t[:], out_offset=None, in_=hext[:],
                        in_offset=bass.IndirectOffsetOnAxis(
                            ap=adidx_sb[:, bb:bb + 1], axis=0),
                    )
                    return t

                adg_next = issue_adg(0)
                for b in range(B):
                    # adg was prefetched one block ahead so the identity
                    # (self-loop) matmul that opens this block's PSUM
                    # accumulation never stalls on the gather queue
                    adg = adg_next
                    if b + 1 < B:
                        adg_next = issue_adg(b + 1)
                    ps_out = pso.tile([P, ncols], f32, tag="psout")
                    # self-loop contribution: source rows == this block's own
                    # nodes == adg; dst one-hot == identity. exp(leaky(a_s+a_d))
                    t_sl = sbs.tile([P, heads], f32, tag="tsl")
                    nc.vector.tensor_add(
                        out=t_sl[:], in0=adg[:, scol:scol + heads],
                        in1=adg[:, scol + heads:scol + 2 * heads])
                    ts2 = sbs.tile([P, heads], f32, tag="tsl2")
                    nc.vector.tensor_scalar_mul(
                        out=ts2[:], in0=t_sl[:], scalar1=NEG_SLOPE)
                    lr_sl = sbs.tile([P, heads], f32, tag="lrsl")
                    nc.vector.tensor_tensor(
                        out=lr_sl[:], in0=t_sl[:], in1=ts2[:], op=Alu.max)
                    rhs_sl = sbe.tile([P, ncols], f32, tag="rhssl")
                    nc.scalar.activation(
                        out=rhs_sl[:, scol:scol + heads], in_=lr_sl[:], func=Act.Exp)
                    for h in range(heads):
                        nc.vector.tensor_scalar_mul(
                            out=rhs_sl[:, h * cdim:(h + 1) * cdim],
                            in0=adg[:, h * cdim:(h + 1) * cdim],
                            scalar1=rhs_sl[:, scol + h:scol + h + 1],
                        )
                    nc.tensor.matmul(
                        out=ps_out[:], lhsT=ident_sb[:], rhs=rhs_sl[:],
                        start=True, stop=(t_b == 0),
                    )
                    for t in range(t_b):
                        j = b * t_b + t
                        g = sbe.tile([P, wcols], f32, tag="g")
                        nc.gpsimd.indirect_dma_start(
                            out=g[:], out_offset=None, in_=hext[:],
                            in_offset=bass.IndirectOffsetOnAxis(
                                ap=srcidx_sb[:, j:j + 1], axis=0),
                        )
                        s01 = sbe.tile([P, P], f32, tag="s01")
                        nc.vector.tensor_scalar(
                            out=s01[:], in0=iota_sb[:],
                            scalar1=dstcol_sb[:, j:j + 1], scalar2=None,
                            op0=Alu.is_equal,
                        )
                        ps_t = pst.tile([P, P], f32, tag="pst")
                        nc.tensor.transpose(out=ps_t[:], in_=s01[:], identity=ident_sb[:])
                        s01t = sbe.tile([P, P], f32, tag="s01t")
                        nc.vector.tensor_copy(out=s01t[:], in_=ps_t[:])
                        ps_e = pse.tile([P, heads], f32, tag="pse")
                        nc.tensor.matmul(
                            out=ps_e[:], lhsT=s01t[:],
                            rhs=adg[:, scol + heads:scol + 2 * heads],
                            start=True, stop=False,
                        )
                        # accumulate a_s[src] into the same PSUM via an
                        # identity matmul: t = a_d[dst] + a_s[src] lands in
                        # ps_e with no DVE add on the critical path
                        nc.tensor.matmul(
                            out=ps_e[:], lhsT=ident_sb[:],
                            rhs=g[:, scol:scol + heads],
                            start=False, stop=True,
                        )
                        rhs = sbe.tile([P, ncols], f32, tag="rhs")
                        # leaky = max(t, slope*t) on the DVE (the ACT Lrelu
                        # table has a hardwired 0.01 slope)
                        ts_sb = sbs.tile([P, heads], f32, tag="tssb")
                        nc.vector.tensor_scalar_mul(
                            out=ts_sb[:], in0=ps_e[:], scalar1=NEG_SLOPE)
                        lr = sbs.tile([P, heads], f32, tag="lr")
                        nc.vector.tensor_tensor(
                            out=lr[:], in0=ps_e[:], in1=ts_sb[:], op=Alu.max)
                        nc.scalar.activation(
                            out=rhs[:, scol:scol + heads], in_=lr[:],
                            func=Act.Exp,
                        )
                        for h in range(heads):
                            nc.vector.tensor_scalar_mul(
                                out=rhs[:, h * cdim:(h + 1) * cdim],
                                in0=g[:, h * cdim:(h + 1) * cdim],
                                scalar1=rhs[:, scol + h:scol + h + 1],
                            )
                        nc.tensor.matmul(
                            out=ps_out[:], lhsT=s01[:], rhs=rhs[:],
                            start=False, stop=(t == t_b - 1),
                        )
                    # ---- block epilogue ----
                    den = sbs.tile([P, heads], f32, tag="den")
                    nc.vector.tensor_scalar_add(
                        out=den[:], in0=ps_out[:, scol:scol + heads], scalar1=DEN_EPS)
                    rec = sbs.tile([P, heads], f32, tag="rec")
                    nc.vector.reciprocal(out=rec[:], in_=den[:])
                    o = sbe.tile([P, scol], f32, tag="o")
                    for h in range(heads):
                        nc.vector.tensor_scalar_mul(
                            out=o[:, h * cdim:(h + 1) * cdim],
                            in0=ps_out[:, h * cdim:(h + 1) * cdim],
                            scalar1=rec[:, h:h + 1],
                        )
                    nc.vector.tensor_add(out=o[:], in0=o[:], in1=bias_sb[:])
                    if layer1:
                        neg = sbe.tile([P, scol], f32, tag="neg")
                        nc.vector.tensor_scalar_min(out=neg[:], in0=o[:], scalar1=0.0)
                        pos = sbe.tile([P, scol], f32, tag="pos")
                        nc.vector.tensor_tensor(
                            out=pos[:], in0=o[:], in1=neg[:], op=Alu.subtract)
                        expm = sbe.tile([P, scol], f32, tag="expm")
                        nc.scalar.activation(out=expm[:], in_=neg[:], func=Act.Exp)
                        em1 = sbe.tile([P, scol], f32, tag="em1")
                        nc.vector.tensor_scalar(
                            out=em1[:], in0=expm[:], scalar1=1.0, scalar2=None,
                            op0=Alu.subtract)
                        h1sb = sbe.tile([P, scol], f32, tag="h1sb")
                        nc.vector.tensor_add(out=h1sb[:], in0=em1[:], in1=pos[:])
                        ps_tr = pst.tile([P, P], f32, tag="pst")
                        nc.tensor.transpose(out=ps_tr[:], in_=h1sb[:], identity=ident_sb[:])
                        h1t = sbe.tile([P, P], f32, tag="h1t")
                        nc.vector.tensor_copy(out=h1t[:], in_=ps_tr[:])
                        nc.sync.dma_start(
                            out=h1t_shard[:, b * P:(b + 1) * P], in_=h1t[:])
                    else:
                        nc.sync.dma_start(
                            out=out2[b * P:(b + 1) * P, :], in_=o[:])

        # ---- layer 1 ----
        with nc.named_scope("p1"):
            phase1(lambda s: xT[:, s * 1024:(s + 1) * 1024], w1_sb, W1C, hext1, 8, NPAD // (8 * P), W1C)
        with nc.named_scope("e1"):
            edge_phase(hext1, W1C, H1, C1, b1_sb, layer1=True)

        # ---- exchange h1^T shards ----
        nc.gpsimd.collective_compute(
            "AllGather",
            mybir.AluOpType.bypass,
            replica_groups=[list(range(NCORES))],
            ins=[h1t_shard.opt()],
            outs=[h1t_full.opt()],
        )

        # ---- layer 2 ----
        def l2_src_view(s):
            d, sj = divmod(s, 14)
            return h1t_full[d * P:(d + 1) * P, sj * 896:(sj + 1) * 896]
        with nc.named_scope("p2"):
            phase1(l2_src_view, w2_sb, W2C, hext2, 7, NCORES * 14, P)
        with nc.named_scope("e2"):
            edge_phase(hext2, P, 1, C2, b2_sb, layer1=False)

    _split_overloaded_waits(nc)
    return nc


def _split_overloaded_waits(nc):
    """This walrus build accepts one sem wait per instruction; hoist extras
    onto NoOps spliced immediately before (same engine => same ordering)."""
    from concourse import mybir
    n_fix = 0
    for bb in nc.main_func.blocks:
        insts = bb.instructions
        out = []
        for ins in insts:
            si = getattr(ins, "sync_info", None)
            waits = list(si.on_wait) if (si and si.on_wait) else []
            if len(waits) > 1:
                si.on_wait = waits[-1:]
                rest = waits[:-1]
                while rest:
                    nop = mybir.InstNoOp(name=f"wsplit-{nc.next_id()}", ins=[], outs=[])
                    nop.engine = ins.engine
                    nop.sync_info = mybir.SyncInfo(on_wait=rest[:1], on_update=[])
                    rest = rest[1:]
                    out.append(nop)
                n_fix += 1
            out.append(ins)
        if len(out) != len(insts):
            insts.clear()
            insts.extend(out)
    return n_fix


# ---------------- entry point ----------------------------------------------
_LAST_EXEC_NS = None
_LAST_SCOPES = None


def kernel(x, edge_index, W1, att_src1, att_dst1, b1, W2, att_src2, att_dst2, b2,
           _trace=False):
    global _LAST_EXEC_NS
    _ensure_axon_hooks()
    import concourse.bass_utils as bass_utils
    bass_utils.upload_artifacts = lambda tmpdir: tmpdir  # no network upload
    from concourse.bass_utils import run_bass_kernel_spmd

    x = np.asarray(x, np.float32)
    edge_index = np.asarray(edge_index)
    W1 = np.asarray(W1, np.float32)
    W2 = np.asarray(W2, np.float32)
    b1 = np.asarray(b1, np.float32)
    b2 = np.asarray(b2, np.float32)

    As1, Ad1 = _att_mat(np.asarray(att_src1, np.float32), np.asarray(att_dst1, np.float32), F_IN)
    As2, Ad2 = _att_mat(np.asarray(att_src2, np.float32), np.asarray(att_dst2, np.float32), C2)
    w1cat = np.concatenate([W1, W1 @ As1, W1 @ Ad1], axis=1).astype(np.float32)
    w2cat = np.concatenate([W2, W2 @ As2, W2 @ Ad2], axis=1).astype(np.float32)

    xT = np.zeros((P, NPAD), np.float32)
    xT[:, :N] = x.T

    src_tiles, dst_tiles, t_b = _prep_edges(edge_index)
    adidx = np.empty((NCORES, P, B), np.int32)
    for d in range(NCORES):
        adidx[d] = d * NS + np.arange(B)[None, :] * P + np.arange(P)[:, None]

    iota_rows = np.tile(np.arange(P, dtype=np.float32), (P, 1))
    ident = np.eye(P, dtype=np.float32)
    b1r = b1.reshape(1, HC1)
    b2r = b2.reshape(1, C2)

    nc = _build_program(t_b)
    in_maps = []
    for d in range(NCORES):
        in_maps.append(dict(
            xT=xT, w1cat=w1cat, w2cat=w2cat, b1row=b1r, b2row=b2r,
            iota_rows=iota_rows, ident=ident,
            srcidx=np.ascontiguousarray(src_tiles[d]),
            dstcol=np.ascontiguousarray(dst_tiles[d]),
            adidx=np.ascontiguousarray(adidx[d]),
        ))
    res = run_bass_kernel_spmd(nc, in_maps, list(range(NCORES)), trace=_trace)
    _LAST_EXEC_NS = res.exec_time_ns
    global _LAST_SCOPES
    _LAST_SCOPES = res.per_core_scope_times
    out = np.concatenate([res.results[d]["out2"] for d in range(NCORES)], axis=0)
    return np.ascontiguousarray(out[:N])
